# revision 2
# baseline (speedup 1.0000x reference)
"""Trainium2 Bass kernel: batched complex-waveform similarity.

Math: reference computes
    bank = ifft_ortho(freq)                # [T, L] complex
    score = rx @ conj(bank).T              # [B, T] complex
    sim   = (score.re^2 + score.im^2) / temperature

Since the ortho DFT is unitary,  score = fft_ortho(rx) @ conj(freq).T.
So the kernel never builds the bank: it DFTs rx via a 128x128 bf16
matmul, then runs one big complex GEMM [B,L]x[L,T] in bf16 with fp32
PSUM accumulation, and a fused squared-magnitude epilogue.  The
1/temperature scale is folded into the DFT matrix host-side (score
scales by 1/sqrt(temp), sim by 1/temp), so the epilogue is exactly
sq(Sr) + Si^2 with no extra scale op.

Sharding: data-parallel over the rx batch dim across 8 NeuronCores;
freq (as a transposed bf16 [L, T] pair) is replicated on every core.

Per-core engine pipeline (main phase is PE-bound, ~216ns per matmul):
  PE   : DFT (bf16) + 512 bf16 matmuls [128,128]@[128,512] -> PSUM Sr/Si
  ACT  : t2 = Square(Si)                    (PSUM -> SBUF)
  DVE  : out = Sr^2 + t2                    (custom fused DVE op, 1cyc/elem)
  SP/ACT: HWDGE DMAs in/out on both rings
"""

import numpy as np
import ml_dtypes

B = 8192
T = 8192
L = 128
NCORES = 8
BPC = B // NCORES  # batch rows per core

_BF16 = ml_dtypes.bfloat16

_CACHE = {}


# --------------------------------------------------------------------------- #
# Custom DVE op: out = Src0^2 + Src1   (2 ALU ops -> 1 cycle/elem)
# (Src0 = Sr from PSUM, Src1 = Si^2 staged by ScalarE)
# --------------------------------------------------------------------------- #
def _get_sqadd_op():
    import concourse.dve_ops as dve_ops
    from concourse.dve_spec import Spec, Src0, Src1, sq, lower, _has_src1
    from concourse.dve_uop import DveOpSpec

    name = "SQ_ADD2_ANT"
    for op in dve_ops.OPS:
        if op.name == name:
            return op

    spec = Spec(
        body=sq(Src0) + Src1,
        reference=lambda in0, in1, s0, s1, imm2: (
            in0.astype(np.float32) ** 2 + in1.astype(np.float32)
        ).astype(np.float32),
    )
    opcode = dve_ops._CUSTOM_DVE_ROW_BASE + len(dve_ops.OPS)
    assert opcode < 0x20
    shas = {}
    for ver in ("v3", "v4"):
        compiled = DveOpSpec(
            name=name, opcode=opcode, uops=lower(spec, ver=ver), rd1_en=_has_src1(spec)
        )
        shas[ver] = compiled.sha(ver)
    op = dve_ops.DveOp(name, spec, subdim=False, uops_sha=shas)
    dve_ops.OPS.append(op)
    dve_ops.CUSTOM_DVE_SPECS[name] = spec
    dve_ops._SUB_OPCODE_FOR_NAME[name] = opcode
    return op


# --------------------------------------------------------------------------- #
# Bass program (one SPMD NeuronCore)
# --------------------------------------------------------------------------- #
def build_nc(bpc=BPC, t=T, debug=False):
    from contextlib import ExitStack

    import concourse.bacc as bacc
    import concourse.bass as bass
    import concourse.mybir as mybir
    import concourse.tile as tile

    f32 = mybir.dt.float32
    bf16 = mybir.dt.bfloat16
    sqadd = _get_sqadd_op()

    NG = 512   # output columns per PSUM group (1 bank)
    FG = 1024  # freq columns per SBUF tile / DMA
    OBW = 2048  # out staging tile width: 4 groups, 8KB rows per DMA
    assert bpc % 128 == 0 and t % FG == 0

    nc = bacc.Bacc("TRN2", target_bir_lowering=False, debug=debug, num_devices=NCORES)

    rxt_r = nc.dram_tensor("rxt_r", [L, bpc], bf16, kind="ExternalInput")
    rxt_i = nc.dram_tensor("rxt_i", [L, bpc], bf16, kind="ExternalInput")
    fqt_r = nc.dram_tensor("fqt_r", [L, t], bf16, kind="ExternalInput")
    fqt_i = nc.dram_tensor("fqt_i", [L, t], bf16, kind="ExternalInput")
    w_r = nc.dram_tensor("w_r", [L, L], bf16, kind="ExternalInput")
    w_i = nc.dram_tensor("w_i", [L, L], bf16, kind="ExternalInput")
    w_ni = nc.dram_tensor("w_ni", [L, L], bf16, kind="ExternalInput")
    out = nc.dram_tensor("out", [bpc, t], f32, kind="ExternalOutput")

    with tile.TileContext(nc) as tc, ExitStack() as ctx:
        consts = ctx.enter_context(tc.tile_pool(name="consts", bufs=1))
        psum = ctx.enter_context(
            tc.tile_pool(name="psum", bufs=4, space=bass.MemorySpace.PSUM)
        )
        sq_pool = ctx.enter_context(tc.tile_pool(name="sq", bufs=6))
        out_pool = ctx.enter_context(tc.tile_pool(name="ob", bufs=4))

        # ---- input DMA triggers first ---------------------------------- #
        # rx halves split across the two HWDGE rings (SP + ScalarE) so rx
        # lands in ~2us; then the DFT consts; then freq pairs split fr->SP,
        # fi->ScalarE so each group's pair completes at the same depth.
        rxr_sb = consts.tile([L, bpc], bf16)
        nc.sync.dma_start(rxr_sb[:], rxt_r[:, :])
        rxi_sb = consts.tile([L, bpc], bf16)
        nc.scalar.dma_start(rxi_sb[:], rxt_i[:, :])
        wr_sb = consts.tile([L, L], bf16)
        nc.sync.dma_start(wr_sb[:], w_r[:, :])
        wni_sb = consts.tile([L, L], bf16)
        nc.sync.dma_start(wni_sb[:], w_ni[:, :])
        wi_sb = consts.tile([L, L], bf16)
        nc.scalar.dma_start(wi_sb[:], w_i[:, :])
        fr_sb = []
        fi_sb = []
        for g in range(t // FG):
            gs = slice(g * FG, (g + 1) * FG)
            ftr = consts.tile([L, FG], bf16, tag=f"fr{g}")
            nc.sync.dma_start(ftr[:], fqt_r[:, gs])
            fti = consts.tile([L, FG], bf16, tag=f"fi{g}")
            nc.scalar.dma_start(fti[:], fqt_i[:, gs])
            fr_sb.append(ftr)
            fi_sb.append(fti)

        # ---- PE warmup -------------------------------------------------- #
        # Dependency-free matmuls ramp the HAM clock gate while rx loads.
        warm_w = consts.tile([128, 128], bf16)
        nc.gpsimd.memset(warm_w[:], 0)
        warm_ps = psum.tile([128, NG], mybir.dt.float32, tag="si")
        for _ in range(24):
            nc.tensor.matmul(warm_ps[:, 0:128], warm_w[:], warm_w[:], start=True, stop=True)

        # ---- DFT of rx (bf16): rxfT = W' @ rxT -------------------------- #
        # W' = ortho DFT matrix / sqrt(temp), symmetric, so PE lhsT is W'.
        # rxfT_r = Wr@rxT_r - Wi@rxT_i ; rxfT_i = Wr@rxT_i + Wi@rxT_r
        rxf_r = consts.tile([L, bpc], bf16)
        rxf_i = consts.tile([L, bpc], bf16)
        rxf_nr = consts.tile([L, bpc], bf16)  # -rxfT_r
        for c0 in range(0, bpc, 512):
            cw = min(512, bpc - c0)
            cs = slice(c0, c0 + cw)
            pr = psum.tile([128, NG], mybir.dt.float32, tag="sr")
            nc.tensor.matmul(pr[:, 0:cw], wr_sb[:], rxr_sb[:, cs], start=True, stop=False)
            nc.tensor.matmul(pr[:, 0:cw], wni_sb[:], rxi_sb[:, cs], start=False, stop=True)
            pi = psum.tile([128, NG], mybir.dt.float32, tag="si")
            nc.tensor.matmul(pi[:, 0:cw], wr_sb[:], rxi_sb[:, cs], start=True, stop=False)
            nc.tensor.matmul(pi[:, 0:cw], wi_sb[:], rxr_sb[:, cs], start=False, stop=True)
            # chunked casts: the first main matmuls only need the first
            # 128-column slice of rxf, so don't gate them on the full cast
            for k0 in range(0, cw, 256):
                ks = slice(c0 + k0, c0 + k0 + 256)
                kp = slice(k0, k0 + 256)
                nc.vector.tensor_copy(rxf_r[:, ks], pr[:, kp])
                nc.vector.tensor_copy(rxf_i[:, ks], pi[:, kp])
                nc.vector.tensor_scalar_mul(rxf_nr[:, ks], pr[:, kp], -1.0)

        # ---- main complex GEMM + fused |.|^2 epilogue ------------------- #
        # Sr = rxf_r.T @ fr + rxf_i.T @ fi
        # Si = rxf_i.T @ fr - rxf_r.T @ fi
        for m in range(bpc // 128):
            ms = slice(m * 128, (m + 1) * 128)
            last_m = m == bpc // 128 - 1
            ob = None
            for n in range(t // NG):
                g, j = divmod(n, FG // NG)
                js = slice(j * NG, (j + 1) * NG)
                sr = psum.tile([128, NG], mybir.dt.float32, tag="sr")
                si = psum.tile([128, NG], mybir.dt.float32, tag="si")
                nc.tensor.matmul(sr[:], rxf_r[:, ms], fr_sb[g][:, js], start=True, stop=False)
                nc.tensor.matmul(sr[:], rxf_i[:, ms], fi_sb[g][:, js], start=False, stop=True)
                nc.tensor.matmul(si[:], rxf_i[:, ms], fr_sb[g][:, js], start=True, stop=False)
                nc.tensor.matmul(si[:], rxf_nr[:, ms], fi_sb[g][:, js], start=False, stop=True)
                t2 = sq_pool.tile([128, NG], f32)
                nc.scalar.square(t2[:], si[:])
                o = n % (OBW // NG)
                if o == 0:
                    ob = out_pool.tile([128, OBW], f32)
                nc.vector._custom_dve(
                    sqadd,
                    out=ob[:, o * NG : (o + 1) * NG],
                    in0=sr[:],
                    in1=t2[:],
                )
                if last_m:
                    # final row: per-group DMAs on alternating rings so the
                    # kernel-exit barrier waits on small transfers only
                    oeng = nc.sync if n % 2 == 0 else nc.scalar
                    oeng.dma_start(
                        out[ms, n * NG : (n + 1) * NG], ob[:, o * NG : (o + 1) * NG]
                    )
                elif o == OBW // NG - 1:
                    n0 = n - o
                    # alternate big output DMAs across both HWDGE rings
                    pair = (m * (t // NG) + n) // (OBW // NG)
                    oeng = nc.scalar if pair % 2 == 0 else nc.sync
                    oeng.dma_start(out[ms, n0 * NG : n0 * NG + OBW], ob[:])

    nc.compile()
    return nc


def _host_prep(rx_real, rx_imag, freq_real, freq_imag, temperature, bpc=BPC, t=T):
    """Layout marshaling only: shard/transpose/cast inputs for the cores."""
    lk = np.outer(np.arange(L), np.arange(L)).astype(np.float64)
    w = np.exp(-2j * np.pi * lk / L) / np.sqrt(L)  # ortho DFT matrix (symmetric)
    # fold the temperature scale into the DFT matrix: sim scales by 1/temp
    w = w / np.sqrt(np.float64(np.asarray(temperature)))
    w_r = np.ascontiguousarray(w.real.astype(np.float32).astype(_BF16))
    w_i = np.ascontiguousarray(w.imag.astype(np.float32).astype(_BF16))
    w_ni = np.ascontiguousarray(-w_i)

    fqt_r = np.ascontiguousarray(freq_real[:t].T.astype(_BF16))  # [L, T]
    fqt_i = np.ascontiguousarray(freq_imag[:t].T.astype(_BF16))

    rxt_r = np.asarray(rx_real, np.float32).T.astype(_BF16)  # [L, B]
    rxt_i = np.asarray(rx_imag, np.float32).T.astype(_BF16)

    in_maps = []
    for c in range(NCORES):
        cs = slice(c * bpc, (c + 1) * bpc)
        in_maps.append(
            {
                "rxt_r": np.ascontiguousarray(rxt_r[:, cs]),
                "rxt_i": np.ascontiguousarray(rxt_i[:, cs]),
                "fqt_r": fqt_r,
                "fqt_i": fqt_i,
                "w_r": w_r,
                "w_i": w_i,
                "w_ni": w_ni,
            }
        )
    return in_maps


def kernel(rx_real, rx_imag, freq_real, freq_imag, temperature):
    from concourse.bass_utils import run_bass_kernel_spmd

    if "nc" not in _CACHE:
        _CACHE["nc"] = build_nc()
    nc = _CACHE["nc"]

    in_maps = _host_prep(rx_real, rx_imag, freq_real, freq_imag, temperature)
    res = run_bass_kernel_spmd(nc, in_maps, core_ids=list(range(NCORES)))
    _CACHE["last_result"] = res
    return np.concatenate([r["out"] for r in res.results], axis=0)


# revision 8
# speedup vs baseline: 1.0239x; 1.0239x over previous
"""Trainium2 Bass kernel: batched complex-waveform similarity.

Math: reference computes
    bank = ifft_ortho(freq)                # [T, L] complex
    score = rx @ conj(bank).T              # [B, T] complex
    sim   = (score.re^2 + score.im^2) / temperature

Since the ortho DFT is unitary,  score = fft_ortho(rx) @ conj(freq).T.
So the kernel never builds the bank: it DFTs rx via a 128x128 bf16
matmul, then runs one big complex GEMM [B,L]x[L,T] in bf16 with fp32
PSUM accumulation, and a fused squared-magnitude epilogue.  The
1/temperature scale is folded into the DFT matrix host-side (score
scales by 1/sqrt(temp), sim by 1/temp), so the epilogue is exactly
sq(Sr) + Si^2 with no extra scale op.

Sharding: data-parallel over the rx batch dim across 8 NeuronCores;
freq (as a transposed bf16 [L, T] pair) is replicated on every core.

Per-core engine pipeline (main phase is PE-bound, ~216ns per matmul):
  PE   : DFT (bf16) + 512 bf16 matmuls [128,128]@[128,512] -> PSUM Sr/Si
  ACT  : t2 = Square(Si)                    (PSUM -> SBUF)
  DVE  : out = Sr^2 + t2                    (custom fused DVE op, 1cyc/elem)
  SP/ACT: HWDGE DMAs in/out on both rings
"""

import numpy as np
import ml_dtypes

B = 8192
T = 8192
L = 128
NCORES = 8
BPC = B // NCORES  # batch rows per core

_BF16 = ml_dtypes.bfloat16

_CACHE = {}


# --------------------------------------------------------------------------- #
# Custom DVE op: out = Src0^2 + Src1   (2 ALU ops -> 1 cycle/elem)
# (Src0 = Sr from PSUM, Src1 = Si^2 staged by ScalarE)
# --------------------------------------------------------------------------- #
def _get_sqadd_op():
    import concourse.dve_ops as dve_ops
    from concourse.dve_spec import Spec, Src0, Src1, sq, lower, _has_src1
    from concourse.dve_uop import DveOpSpec

    name = "SQ_ADD2_ANT"
    for op in dve_ops.OPS:
        if op.name == name:
            return op

    spec = Spec(
        body=sq(Src0) + Src1,
        reference=lambda in0, in1, s0, s1, imm2: (
            in0.astype(np.float32) ** 2 + in1.astype(np.float32)
        ).astype(np.float32),
    )
    opcode = dve_ops._CUSTOM_DVE_ROW_BASE + len(dve_ops.OPS)
    assert opcode < 0x20
    shas = {}
    for ver in ("v3", "v4"):
        compiled = DveOpSpec(
            name=name, opcode=opcode, uops=lower(spec, ver=ver), rd1_en=_has_src1(spec)
        )
        shas[ver] = compiled.sha(ver)
    op = dve_ops.DveOp(name, spec, subdim=False, uops_sha=shas)
    dve_ops.OPS.append(op)
    dve_ops.CUSTOM_DVE_SPECS[name] = spec
    dve_ops._SUB_OPCODE_FOR_NAME[name] = opcode
    return op


# --------------------------------------------------------------------------- #
# Bass program (one SPMD NeuronCore)
# --------------------------------------------------------------------------- #
def build_nc(bpc=BPC, t=T, debug=False):
    from contextlib import ExitStack

    import concourse.bacc as bacc
    import concourse.bass as bass
    import concourse.mybir as mybir
    import concourse.tile as tile

    f32 = mybir.dt.float32
    bf16 = mybir.dt.bfloat16
    sqadd = _get_sqadd_op()

    NG = 512   # output columns per PSUM group (1 bank)
    FG = 1024  # freq columns per SBUF tile / DMA
    OBW = 2048  # out staging tile width: 4 groups, 8KB rows per DMA
    assert bpc % 128 == 0 and t % FG == 0

    nc = bacc.Bacc("TRN2", target_bir_lowering=False, debug=debug, num_devices=NCORES)

    # packed inputs: 4KB DRAM rows -> full-size DMA packets
    rxp = nc.dram_tensor("rxp", [L, 2 * bpc], bf16, kind="ExternalInput")
    fqp = nc.dram_tensor("fqp", [L, 2 * t], bf16, kind="ExternalInput")
    wp = nc.dram_tensor("wp", [L, 3 * L], bf16, kind="ExternalInput")
    out = nc.dram_tensor("out", [bpc, t], f32, kind="ExternalOutput")

    with tile.TileContext(nc) as tc, ExitStack() as ctx:
        consts = ctx.enter_context(tc.tile_pool(name="consts", bufs=1))
        psum = ctx.enter_context(
            tc.tile_pool(name="psum", bufs=4, space=bass.MemorySpace.PSUM)
        )
        sq_pool = ctx.enter_context(tc.tile_pool(name="sq", bufs=6))
        out_pool = ctx.enter_context(tc.tile_pool(name="ob", bufs=4))

        # ---- input DMA triggers first ---------------------------------- #
        # Everything packed to 4KB DRAM rows.  rx (one 512KB DMA) on the SP
        # ring; W + freq group 0 lead the ScalarE ring; remaining freq
        # groups (512KB [fr_g|fi_g] pairs) alternate across both rings.
        rx_sb = consts.tile([L, 2 * bpc], bf16)
        nc.sync.dma_start(rx_sb[:], rxp[:, :])
        w_sb = consts.tile([L, 3 * L], bf16)
        nc.scalar.dma_start(w_sb[:], wp[:, :])
        fq_sb = []
        for g in range(t // FG):
            gs = slice(g * 2 * FG, (g + 1) * 2 * FG)
            fq = consts.tile([L, 2 * FG], bf16, tag=f"fq{g}")
            eng = nc.scalar if g % 2 == 0 else nc.sync
            eng.dma_start(fq[:], fqp[:, gs])
            fq_sb.append(fq)

        # ---- PE warmup -------------------------------------------------- #
        # Dependency-free matmuls ramp the HAM clock gate while rx loads.
        warm_w = consts.tile([128, 128], bf16)
        nc.gpsimd.memset(warm_w[:], 0)
        warm_ps = psum.tile([128, NG], mybir.dt.float32, tag="si")
        for _ in range(24):
            nc.tensor.matmul(warm_ps[:, 0:128], warm_w[:], warm_w[:], start=True, stop=True)

        # ---- DFT of rx (bf16): rxfT = W' @ rxT -------------------------- #
        # W' = ortho DFT matrix / sqrt(temp), symmetric, so PE lhsT is W'.
        # rxfT_r = Wr@rxT_r - Wi@rxT_i ; rxfT_i = Wr@rxT_i + Wi@rxT_r
        rxf_r = consts.tile([L, bpc], bf16)
        rxf_i = consts.tile([L, bpc], bf16)
        rxf_nr = consts.tile([L, bpc], bf16)  # -rxfT_r
        for c0 in range(0, bpc, 512):
            cw = min(512, bpc - c0)
            cr = slice(c0, c0 + cw)           # rx_real cols in rx_sb
            ci = slice(bpc + c0, bpc + c0 + cw)  # rx_imag cols in rx_sb
            wr = slice(0, L)
            wni = slice(L, 2 * L)
            wi = slice(2 * L, 3 * L)
            pr = psum.tile([128, NG], mybir.dt.float32, tag="sr")
            nc.tensor.matmul(pr[:, 0:cw], w_sb[:, wr], rx_sb[:, cr], start=True, stop=False)
            nc.tensor.matmul(pr[:, 0:cw], w_sb[:, wni], rx_sb[:, ci], start=False, stop=True)
            pi = psum.tile([128, NG], mybir.dt.float32, tag="si")
            nc.tensor.matmul(pi[:, 0:cw], w_sb[:, wr], rx_sb[:, ci], start=True, stop=False)
            nc.tensor.matmul(pi[:, 0:cw], w_sb[:, wi], rx_sb[:, cr], start=False, stop=True)
            # chunked casts: the first main matmuls only need the first
            # 128-column slice of rxf, so don't gate them on the full cast
            for k0 in range(0, cw, 256):
                ks = slice(c0 + k0, c0 + k0 + 256)
                kp = slice(k0, k0 + 256)
                nc.vector.tensor_copy(rxf_r[:, ks], pr[:, kp])
                nc.vector.tensor_copy(rxf_i[:, ks], pi[:, kp])
                nc.vector.tensor_scalar_mul(rxf_nr[:, ks], pr[:, kp], -1.0)

        # ---- main complex GEMM + fused |.|^2 epilogue ------------------- #
        # Sr = rxf_r.T @ fr + rxf_i.T @ fi
        # Si = rxf_i.T @ fr - rxf_r.T @ fi
        for m in range(bpc // 128):
            ms = slice(m * 128, (m + 1) * 128)
            last_m = m == bpc // 128 - 1
            ob = None
            for n in range(t // NG):
                g, j = divmod(n, FG // NG)
                jr = slice(j * NG, (j + 1) * NG)           # fr cols in fq tile
                ji = slice(FG + j * NG, FG + (j + 1) * NG)  # fi cols in fq tile
                fq = fq_sb[g]
                sr = psum.tile([128, NG], mybir.dt.float32, tag="sr")
                si = psum.tile([128, NG], mybir.dt.float32, tag="si")
                nc.tensor.matmul(sr[:], rxf_r[:, ms], fq[:, jr], start=True, stop=False)
                nc.tensor.matmul(sr[:], rxf_i[:, ms], fq[:, ji], start=False, stop=True)
                nc.tensor.matmul(si[:], rxf_i[:, ms], fq[:, jr], start=True, stop=False)
                nc.tensor.matmul(si[:], rxf_nr[:, ms], fq[:, ji], start=False, stop=True)
                t2 = sq_pool.tile([128, NG], f32)
                nc.scalar.square(t2[:], si[:])
                o = n % (OBW // NG)
                if o == 0:
                    ob = out_pool.tile([128, OBW], f32)
                nc.vector._custom_dve(
                    sqadd,
                    out=ob[:, o * NG : (o + 1) * NG],
                    in0=sr[:],
                    in1=t2[:],
                )
                if last_m:
                    # final row: per-group DMAs on alternating rings so the
                    # kernel-exit barrier waits on small transfers only
                    oeng = nc.sync if n % 2 == 0 else nc.scalar
                    oeng.dma_start(
                        out[ms, n * NG : (n + 1) * NG], ob[:, o * NG : (o + 1) * NG]
                    )
                elif o == OBW // NG - 1:
                    n0 = n - o
                    # alternate big output DMAs across both HWDGE rings
                    pair = (m * (t // NG) + n) // (OBW // NG)
                    oeng = nc.scalar if pair % 2 == 0 else nc.sync
                    oeng.dma_start(out[ms, n0 * NG : n0 * NG + OBW], ob[:])

    nc.compile()
    return nc


def _host_prep(rx_real, rx_imag, freq_real, freq_imag, temperature, bpc=BPC, t=T):
    """Layout marshaling only: shard/transpose/cast inputs for the cores."""
    FG = 1024
    lk = np.outer(np.arange(L), np.arange(L)).astype(np.float64)
    w = np.exp(-2j * np.pi * lk / L) / np.sqrt(L)  # ortho DFT matrix (symmetric)
    # fold the temperature scale into the DFT matrix: sim scales by 1/temp
    w = w / np.sqrt(np.float64(np.asarray(temperature)))
    w_r = w.real.astype(np.float32).astype(_BF16)
    w_i = w.imag.astype(np.float32).astype(_BF16)
    # packed [wr | -wi | wi], 4KB-class rows
    wp = np.ascontiguousarray(np.concatenate([w_r, -w_i, w_i], axis=1))

    fqt_r = freq_real[:t].T.astype(_BF16)  # [L, T]
    fqt_i = freq_imag[:t].T.astype(_BF16)
    # packed freq: per group g of FG columns, [fr_g | fi_g] -> 4KB rows
    fqp = np.empty((L, 2 * t), _BF16)
    for g in range(t // FG):
        fqp[:, 2 * g * FG : (2 * g + 1) * FG] = fqt_r[:, g * FG : (g + 1) * FG]
        fqp[:, (2 * g + 1) * FG : (2 * g + 2) * FG] = fqt_i[:, g * FG : (g + 1) * FG]
    fqp = np.ascontiguousarray(fqp)

    rxt_r = np.asarray(rx_real, np.float32).T.astype(_BF16)  # [L, B]
    rxt_i = np.asarray(rx_imag, np.float32).T.astype(_BF16)

    in_maps = []
    for c in range(NCORES):
        cs = slice(c * bpc, (c + 1) * bpc)
        rxp = np.ascontiguousarray(
            np.concatenate([rxt_r[:, cs], rxt_i[:, cs]], axis=1)
        )
        in_maps.append({"rxp": rxp, "fqp": fqp, "wp": wp})
    return in_maps


def kernel(rx_real, rx_imag, freq_real, freq_imag, temperature):
    from concourse.bass_utils import run_bass_kernel_spmd

    if "nc" not in _CACHE:
        _CACHE["nc"] = build_nc()
    nc = _CACHE["nc"]

    in_maps = _host_prep(rx_real, rx_imag, freq_real, freq_imag, temperature)
    res = run_bass_kernel_spmd(nc, in_maps, core_ids=list(range(NCORES)))
    _CACHE["last_result"] = res
    return np.concatenate([r["out"] for r in res.results], axis=0)
